# revision 11
# baseline (speedup 1.0000x reference)
"""Trainium2 Bass kernel for nn_ChunkwiseRecurrentAttentionCell.

Math (per (b,h) slice; T=256, Dk=Dv=128):
    gc = cumsum(g);  A = tril(beta_i exp(gc_i-gc_j) k_i.k_j, -1)
    v_new = (I+A)^{-1} (beta v - beta exp(gc) (k @ S0))
    out   = exp(gc) (q@S0) + (tril(exp(gc_i-gc_j),0) * (q k^T)) @ v_new
    S_new = exp(gc_T) S0 + k^T (v_new * exp(gc_T - gc))

Chunked recurrence (2 chunks of 128) keeps all exp ratios <= e^6.4
(fp16-safe).  Triangular solve: 8-term Neumann product form
(I+X^4)(I+X^2)(I+X), X = -A_chunk.

Perf structure (v3): q/k pre-transposed + pre-cast fp16 on host; v
pre-scaled by beta on host.  Gate rows [nbr|r|ir] packed host-side and
partition-broadcast once per slice; the three scaled transposed
operands are built with two wide fp16 DVE tensor_tensor ops per slice.
The three Gram products share one packed PSUM bank and are masked by a
single 384-wide tensor_tensor against a packed mask constant.  PSUM is
packed into 4 banks x 2 bufs (gram / powers / z-chain / out+state) so
two slices can be in flight per stage.  Adds ride either DVE
tensor_tensor (PSUM read + add + fp16 move in one op) or identity-
matmul PSUM accumulation, split to balance DVE vs ACT.

Sharding: (B,H) flattened to 512 slices, 64 per core across 8 cores
(data parallel, no collectives).
"""

import numpy as np

import concourse.bass as bass
import concourse.mybir as mybir
from concourse import bacc
from concourse.tile import TileContext
from concourse.masks import (
    make_identity,
    make_lower_triangular,
    make_upper_triangular,
)

B, H, T, DK, DV = 16, 32, 256, 128, 128
N_CORES = 8
N_SLICES = (B * H) // N_CORES  # 64 per core
CH = 128
N_CHUNKS = T // CH

F32 = mybir.dt.float32
F16 = mybir.dt.float16

_ALU = mybir.AluOpType
_ACTF = mybir.ActivationFunctionType


def build_nc(n_slices: int = N_SLICES):
    nc = bacc.Bacc("TRN2", target_bir_lowering=False)

    dqt = nc.dram_tensor("qT", [n_slices, DK, T], F16, kind="ExternalInput")
    dkt = nc.dram_tensor("kT", [n_slices, DK, T], F16, kind="ExternalInput")
    dk = nc.dram_tensor("k", [n_slices, T, DK], F16, kind="ExternalInput")
    dv = nc.dram_tensor("v", [n_slices, T, DV], F16, kind="ExternalInput")  # beta*v
    # packed gate rows per slice: [nbr(256) | r(256) | ir(256)], partition 0
    drw = nc.dram_tensor("rows", [1, n_slices * 3 * T], F16, kind="ExternalInput")
    det = nc.dram_tensor("eirT", [N_CHUNKS, CH, n_slices], F32, kind="ExternalInput")
    dee = nc.dram_tensor("ET", [N_CHUNKS, CH, n_slices], F32, kind="ExternalInput")
    ds0 = nc.dram_tensor("s0", [n_slices, DK, DV], F16, kind="ExternalInput")
    dout = nc.dram_tensor("out", [n_slices, T, DV], F32, kind="ExternalOutput")
    dsn = nc.dram_tensor("s_new", [n_slices, DK, DV], F32, kind="ExternalOutput")

    with TileContext(nc) as tc:
        with (
            tc.tile_pool(name="const", bufs=1) as cpool,
            tc.tile_pool(name="sl", bufs=8) as slp,
            tc.tile_pool(name="ck", bufs=8) as ckp,
            tc.tile_pool(name="st", bufs=6) as stp,
            tc.tile_pool(name="ps", bufs=1, space="PSUM") as psp,
        ):
            # ---------------- constants ----------------
            ident16 = cpool.tile([128, 128], F16)
            make_identity(nc, ident16)
            # packed mask [su | ui | sl] matching gram bank [at | cq | a]
            mask3 = cpool.tile([128, 3 * CH], F32)
            make_upper_triangular(nc, mask3[:, 0:CH], val=1.0, diag=False)
            make_upper_triangular(nc, mask3[:, CH : 2 * CH], val=1.0, diag=True)
            make_lower_triangular(nc, mask3[:, 2 * CH : 3 * CH], val=1.0, diag=False)

            # ---------------- gate setup (host-precomputed) ----------------
            rows = cpool.tile([1, n_slices * 3 * T], F16)
            nc.sync.dma_start(rows[:], drw[:])
            eirTl, ETl = [], []
            for c in range(N_CHUNKS):
                for src, lst, nm in ((det, eirTl, "eiT"), (dee, ETl, "ET")):
                    col = cpool.tile([CH, n_slices], F32, name=f"{nm}_{c}")
                    nc.sync.dma_start(col[:], src[c, :, :])
                    lst.append(col)

            # ---------------- main loop over slices ----------------
            for s in range(n_slices):
                kq = slp.tile([DK, 2 * T], F16, name="kq")
                nc.sync.dma_start(kq[:, 0:T], dkt[s, :, :])
                nc.sync.dma_start(kq[:, T : 2 * T], dqt[s, :, :])
                B3 = slp.tile([128, 3 * T], F16, name="B3")
                nc.gpsimd.partition_broadcast(
                    B3[:], rows[0:1, s * 3 * T : (s + 1) * 3 * T]
                )
                s_cur = stp.tile([DK, DV], F16, name="s_cur")
                nc.sync.dma_start(s_cur[:], ds0[s, :, :])

                # sc6 = [kTn_full | qTr_full | kTi_full] (chunk-minor)
                sc6 = slp.tile([DK, 3 * T], F16, name="sc6")
                nc.vector.tensor_tensor(
                    sc6[:, 0 : 2 * T], kq[:, 0 : 2 * T], B3[:, 0 : 2 * T], _ALU.mult
                )
                nc.vector.tensor_tensor(
                    sc6[:, 2 * T : 3 * T], kq[:, 0:T], B3[:, 2 * T : 3 * T], _ALU.mult
                )

                for c in range(N_CHUNKS):
                    cs = slice(c * CH, (c + 1) * CH)
                    kTn_c = sc6[:, c * CH : (c + 1) * CH]
                    qTr_c = sc6[:, T + c * CH : T + (c + 1) * CH]
                    kTi_c = sc6[:, 2 * T + c * CH : 2 * T + (c + 1) * CH]

                    k_c = ckp.tile([CH, DK], F16, name="k_c")
                    nc.sync.dma_start(k_c[:], dk[s, cs, :])
                    v_c = ckp.tile([CH, DV], F16, name="v_c")
                    nc.sync.dma_start(v_c[:], dv[s, cs, :])

                    # Gram bank: [at | cq | a]
                    ps_g = psp.tile([CH, 3 * CH], F32, name="ps_g", tag="g", bufs=2)
                    nc.tensor.matmul(ps_g[:, 0:CH], kTi_c, kTn_c)
                    nc.tensor.matmul(ps_g[:, CH : 2 * CH], kTi_c, qTr_c)
                    nc.tensor.matmul(ps_g[:, 2 * CH : 3 * CH], kTn_c, kTi_c)
                    # one masked move: [c0 | cqt | b0]
                    mcc = ckp.tile([CH, 3 * CH], F16, name="mcc")
                    nc.vector.tensor_tensor(mcc[:], ps_g[:], mask3[:], _ALU.mult)
                    c0 = mcc[:, 0:CH]
                    cqt = mcc[:, CH : 2 * CH]
                    b0 = mcc[:, 2 * CH : 3 * CH]

                    # power bank: [b1 | c1 | c2]
                    ps_p = psp.tile([CH, 3 * CH], F32, name="ps_p", tag="p", bufs=2)
                    nc.tensor.matmul(ps_p[:, 0:CH], c0, b0)
                    nc.tensor.matmul(ps_p[:, CH : 2 * CH], b0, c0)
                    bc = ckp.tile([CH, 2 * CH], F16, name="bc")
                    nc.scalar.copy(bc[:], ps_p[:, 0 : 2 * CH])
                    nc.tensor.matmul(ps_p[:, 2 * CH : 3 * CH], bc[:, 0:CH], bc[:, CH : 2 * CH])
                    c2 = ckp.tile([CH, CH], F16, name="c2")
                    nc.scalar.copy(c2[:], ps_p[:, 2 * CH : 3 * CH])

    # z-chain: one short-lived PSUM tile per stage, 2 rotating banks
                    ps_y = psp.tile([CH, DV], F32, name="ps_y", tag="z", bufs=2)
                    nc.tensor.matmul(ps_y[:], kTn_c, s_cur[:])
                    z = ckp.tile([CH, DV], F16, name="z0", tag="z", bufs=12)
                    nc.vector.tensor_tensor(z[:], ps_y[:], v_c[:], _ALU.add)
                    # ap1 (DVE add), ap2 (ident-MM + ACT copy), ap3 (DVE add)
                    ps_a1 = psp.tile([CH, DV], F32, name="ps_a1", tag="z", bufs=2)
                    nc.tensor.matmul(ps_a1[:], c0, z[:])
                    z1 = ckp.tile([CH, DV], F16, name="z1", tag="z", bufs=12)
                    nc.vector.tensor_tensor(z1[:], ps_a1[:], z[:], _ALU.add)
                    ps_a2 = psp.tile([CH, DV], F32, name="ps_a2", tag="z", bufs=2)
                    nc.tensor.matmul(ps_a2[:], bc[:, CH : 2 * CH], z1[:], start=True, stop=False)
                    nc.tensor.matmul(ps_a2[:], ident16[:], z1[:], start=False, stop=True)
                    z2 = ckp.tile([CH, DV], F16, name="z2", tag="z", bufs=12)
                    nc.scalar.copy(z2[:], ps_a2[:])
                    ps_a3 = psp.tile([CH, DV], F32, name="ps_a3", tag="z", bufs=2)
                    nc.tensor.matmul(ps_a3[:], c2[:], z2[:])
                    z3 = ckp.tile([CH, DV], F16, name="z3", tag="z", bufs=12)
                    nc.vector.tensor_tensor(z3[:], ps_a3[:], z2[:], _ALU.add)

                    # out/state bank: [o | s]
                    ps_os = psp.tile([CH, 2 * CH], F32, name="ps_os", tag="os", bufs=2)
                    nc.tensor.matmul(ps_os[:, 0:CH], qTr_c, s_cur[:], start=True, stop=False)
                    nc.tensor.matmul(ps_os[:, 0:CH], cqt, z3[:], start=False, stop=True)
                    o_sb = ckp.tile([CH, DV], F32, name="o_sb")
                    nc.scalar.copy(o_sb[:], ps_os[:, 0:CH])
                    nc.sync.dma_start(dout[s, cs, :], o_sb[:])

                    zs = ckp.tile([CH, DV], F16, name="zs")
                    nc.scalar.activation(
                        zs[:], z3[:], _ACTF.Copy, scale=eirTl[c][:, s : s + 1]
                    )
                    nc.tensor.matmul(ps_os[:, CH : 2 * CH], k_c[:], zs[:])
                    if c < N_CHUNKS - 1:
                        s_nx = stp.tile([DK, DV], F16, name="s_nx")
                        nc.vector.scalar_tensor_tensor(
                            s_nx[:], s_cur[:], ETl[c][:, s : s + 1], ps_os[:, CH : 2 * CH],
                            op0=_ALU.mult, op1=_ALU.add,
                        )
                        s_cur = s_nx
                    else:
                        s_fin = stp.tile([DK, DV], F32, name="s_fin")
                        nc.vector.scalar_tensor_tensor(
                            s_fin[:], s_cur[:], ETl[c][:, s : s + 1], ps_os[:, CH : 2 * CH],
                            op0=_ALU.mult, op1=_ALU.add,
                        )
                        nc.sync.dma_start(dsn[s, :, :], s_fin[:])

    nc.compile()
    return nc


_NC_CACHE = {}


def _get_nc(n_slices):
    if n_slices not in _NC_CACHE:
        _NC_CACHE[n_slices] = build_nc(n_slices)
    return _NC_CACHE[n_slices]


def _prep_inputs(q, k, v, g, beta, last_recurrent_state):
    ns = B * H
    qf = np.asarray(q, np.float32).reshape(ns, T, DK)
    kf = np.asarray(k, np.float32).reshape(ns, T, DK)
    vf = np.asarray(v, np.float32).reshape(ns, T, DV)
    gf = np.asarray(g, np.float32).reshape(ns, T)
    bf = np.asarray(beta, np.float32).reshape(ns, T)
    sf = np.asarray(last_recurrent_state, np.float32).reshape(ns, DK, DV)

    # per-chunk local gate cumsum
    gc = np.cumsum(gf, axis=-1)
    gcl = gc.copy()
    gcl[:, CH:] -= gc[:, CH - 1 : CH]
    r = np.exp(gcl)
    gclc = gcl.reshape(ns, N_CHUNKS, CH)
    glast = gclc[:, :, -1:]                       # [ns, 2, 1]
    eir = np.exp(glast - gclc)                    # [ns, 2, CH]
    eirT = np.ascontiguousarray(eir.transpose(1, 2, 0), np.float32)
    ET = np.ascontiguousarray(
        np.broadcast_to(np.exp(glast).transpose(1, 2, 0), (N_CHUNKS, CH, ns)),
        np.float32,
    )
    rows = np.concatenate([(-bf * r), r, np.exp(-gcl)], axis=1)  # [ns, 3T]
    return {
        "qT": qf.transpose(0, 2, 1).astype(np.float16),
        "kT": kf.transpose(0, 2, 1).astype(np.float16),
        "k": kf.astype(np.float16),
        "v": (bf[:, :, None] * vf).astype(np.float16),
        "rows": rows.astype(np.float16),           # [ns, 3T], flat at shard
        "eirT": eirT,                              # [2, CH, ns], shard last axis
        "ET": ET,
        "s0": sf.astype(np.float16),
    }


def _shard(full, lo, hi):
    m = {}
    for name, arr in full.items():
        if name in ("eirT", "ET"):
            m[name] = np.ascontiguousarray(arr[:, :, lo:hi])
        elif name == "rows":
            m[name] = np.ascontiguousarray(arr[lo:hi]).reshape(1, -1)
        else:
            m[name] = arr[lo:hi]
    return m


def kernel(q, k, v, g, beta, last_recurrent_state):
    from concourse.bass_utils import run_bass_kernel_spmd

    full = _prep_inputs(q, k, v, g, beta, last_recurrent_state)
    nc = _get_nc(N_SLICES)
    in_maps = [
        _shard(full, i * N_SLICES, (i + 1) * N_SLICES) for i in range(N_CORES)
    ]
    res = run_bass_kernel_spmd(nc, in_maps, list(range(N_CORES)))
    out = np.concatenate([res.results[i]["out"] for i in range(N_CORES)], axis=0)
    s_new = np.concatenate([res.results[i]["s_new"] for i in range(N_CORES)], axis=0)
    return np.concatenate([out.reshape(-1), s_new.reshape(-1)], axis=0)
